# revision 13
# baseline (speedup 1.0000x reference)
"""Trainium2 Bass kernel for nn_MultiHeadAttention_28114855920396.

Reference computation (heads folded into sequence):
    xh = x.reshape(b, n*h, d)                      # [2, 8192, 64]
    q/k/v = xh @ w{q,k,v}.T
    attn  = softmax(q @ k.T / sqrt(d))             # [2, 8192, 8192]  <- output 0
    out   = (attn @ v).reshape(b, n, e)
    out   = gelu(out @ w1.T + b1) @ w2.T + b2      # [2, 1024, 512]   <- output 1

Sharding: 8 cores = 2 batches x 4 query-row quarters. Each core computes
attn rows [2048, 8192] for one batch plus the MLP for its 256 token rows.
Everything is done on-device per core; no collectives.

Per-core dataflow:
  pass1: scores^T [t,s] tiles (PE, f32r) -> exp (ACT) -> transient U^T blocks
         -> A.V matmul with [v|ones] stationary (PE) => out_un^T + rowsums r
  tail:  recip(r), r transposed via PE, bias = -ln(r) (ACT)
  pass2: scores [s,t] tiles (PE) -> ACT exp(SCALE*x - ln r) = normalized attn
         -> 4MB DMAs to HBM
  mlp:   PE-transposed w1/w2, gelu with per-partition bias on ACT,
         b2 added via K=1 ones-matmul into the psum accumulation.
"""

import os
import sys

if "/opt/trn_rl_repo" not in sys.path:
    sys.path.insert(0, "/opt/trn_rl_repo")

import numpy as np

NUM_HEADS = 8
EMBED = 512
HD = 64                     # head_dim
SCALE = 1.0 / (HD ** 0.5)   # 0.125
B = 2
N = 1024                    # tokens
S = N * NUM_HEADS           # 8192 folded sequence
NQ = N // 4                 # 256 token rows per core
SC = S // 4                 # 2048 folded query rows per core

# matmul input dtype for the three big attention matmuls:
#   "f32r" - fp32 storage, PE reads as float32r (1 cyc/row, reduced precision)
#   "bf16" - bf16 storage for qT/kT/U^T/v (1 cyc/row)
#   "f32"  - full fp32 (4 cyc/row, slow but exact)
MM_MODE = os.environ.get("BASSMM_DTYPE", "f32r")

_cache = {}


def _build_nc():
    import concourse.bacc as bacc
    import concourse.mybir as mybir
    import concourse.tile as tile
    from concourse import masks

    dt = mybir.dt
    AF = mybir.ActivationFunctionType
    f32 = dt.float32
    if MM_MODE == "bf16":
        mm_store_dt = dt.bfloat16
    elif MM_MODE == "f32r":
        mm_store_dt = dt.float32r
    else:
        mm_store_dt = f32

    def mmt(ap):
        return ap

    nc = bacc.Bacc("TRN2", target_bir_lowering=False)

    x_full = nc.dram_tensor("x_full", [N, EMBED], f32, kind="ExternalInput")
    x_q = nc.dram_tensor("x_q", [NQ, EMBED], f32, kind="ExternalInput")
    wq_t = nc.dram_tensor("wq", [HD, HD], f32, kind="ExternalInput")
    wk_t = nc.dram_tensor("wk", [HD, HD], f32, kind="ExternalInput")
    wv_t = nc.dram_tensor("wv", [HD, HD], f32, kind="ExternalInput")
    w1_t = nc.dram_tensor("w1", [2 * EMBED, EMBED], f32, kind="ExternalInput")
    b1_t = nc.dram_tensor("b1", [2 * EMBED], f32, kind="ExternalInput")
    w2_t = nc.dram_tensor("w2", [EMBED, 2 * EMBED], f32, kind="ExternalInput")
    b2_t = nc.dram_tensor("b2", [EMBED], f32, kind="ExternalInput")
    attn_p = nc.dram_tensor("attn_p", [SC, S], f32, kind="ExternalOutput")
    y_p = nc.dram_tensor("y_p", [NQ, EMBED], f32, kind="ExternalOutput")

    with tile.TileContext(nc) as tc:
        with tc.tile_pool(name="persist", bufs=1) as pp:
            ident = pp.tile([128, 128], f32, tag="ident")
            masks.make_identity(nc, ident[:])
            ones_row = pp.tile([1, 128], f32, tag="ones")
            nc.vector.memset(ones_row[:], 1.0)

            # persistent SBUF operands
            kT = pp.tile([128, S], mm_store_dt, tag="kT")        # duplicated halves
            qT = pp.tile([128, SC], mm_store_dt, tag="qT")       # duplicated halves
            vo = pp.tile([128, 65 * 64], mm_store_dt, tag="vo")  # [v | ones] per t-tile
            outT = pp.tile([64, SC], f32, tag="outT")            # normalized (attn@v)^T
            nlr = pp.tile([128, 16], f32, tag="nlr")             # -ln(rowsum) per s_tile

            # ---------------- phase 0/1: load x, build xh^T ----------------
            with tc.tile_pool(name="early", bufs=1) as ep, \
                 tc.tile_pool(name="early_ps", bufs=2, space="PSUM") as eps:
                xs = ep.tile([128, 64 * 64], f32, tag="xs")
                nc.sync.dma_start(
                    xs.rearrange("p (i d) -> p i d", d=64),
                    x_full.rearrange("(i n) (h d) -> (n h) i d", n=16, h=8))
                xqs = ep.tile([128, 16 * 64], f32, tag="xqs")
                nc.sync.dma_start(
                    xqs.rearrange("p (i d) -> p i d", d=64),
                    x_q.rearrange("(i n) (h d) -> (n h) i d", n=16, h=8))

                xhT = ep.tile([64, S], f32, tag="xhT")
                xqT = ep.tile([64, SC], f32, tag="xqT")
                for base, src, dst, nt in ((0, xs, xhT, 64), (0, xqs, xqT, 16)):
                    for g in range(nt // 4):  # groups of 4 tiles -> one psum bank
                        pt = eps.tile([64, 512], f32, tag="tr")
                        for u in range(4):
                            i = g * 4 + u
                            nc.tensor.transpose(
                                pt[:, u * 128:(u + 1) * 128],
                                src[:, i * 64:(i + 1) * 64], ident[:, :])
                        nc.any.tensor_copy(dst[:, g * 512:(g + 1) * 512], pt[:])

                # small weights -> transposed [d, e] layout
                ws = ep.tile([64, 3 * 64], f32, tag="ws")
                nc.sync.dma_start(ws[:, 0:64], wq_t[:, :])
                nc.sync.dma_start(ws[:, 64:128], wk_t[:, :])
                nc.sync.dma_start(ws[:, 128:192], wv_t[:, :])
                wT = ep.tile([64, 3 * 64], f32, tag="wT")
                for i in range(3):
                    pt = eps.tile([64, 512], f32, tag="tr")
                    nc.tensor.transpose(
                        pt[:, 0:64], ws[:, i * 64:(i + 1) * 64], ident[0:64, 0:64])
                    nc.any.tensor_copy(wT[:, i * 64:(i + 1) * 64], pt[:, 0:64])
                wqT, wkT, wvT = (wT[:, 0:64], wT[:, 64:128], wT[:, 128:192])

                # ---------------- phase 2: projections (fp32, exact) --------
                # kT[e, t] = (wkT).T @ xhT ; duplicated into both partition halves
                for ch in range(16):
                    pk = eps.tile([64, 512], f32, tag="prj")
                    nc.tensor.matmul(pk[:], wkT, xhT[:, ch * 512:(ch + 1) * 512])
                    nc.any.tensor_copy(kT[0:64, ch * 512:(ch + 1) * 512], pk[:])
                    nc.any.tensor_copy(kT[64:128, ch * 512:(ch + 1) * 512], pk[:])
                for ch in range(4):
                    pq = eps.tile([64, 512], f32, tag="prj")
                    nc.tensor.matmul(pq[:], wqT, xqT[:, ch * 512:(ch + 1) * 512])
                    nc.any.tensor_copy(qT[0:64, ch * 512:(ch + 1) * 512], pq[:])
                    nc.any.tensor_copy(qT[64:128, ch * 512:(ch + 1) * 512], pq[:])
                # v natural [t, d] with a ones column appended per 128-row t-tile
                vo3 = vo.rearrange("p (j c) -> p j c", c=65)
                ones_col = ep.tile([128, 1], f32, tag="ones_col")
                nc.vector.memset(ones_col[:], 1.0)
                nc.vector.tensor_copy(
                    vo3[:, :, 64:65], ones_col[:].broadcast_to((128, 64, 1)))
                for g in range(8):  # 8 t-tiles per psum bank
                    pv = eps.tile([128, 512], f32, tag="prj")
                    for u in range(8):
                        j = g * 8 + u
                        nc.tensor.matmul(
                            pv[:, u * 64:(u + 1) * 64],
                            xhT[:, j * 128:(j + 1) * 128], wvT)
                    nc.any.tensor_copy(
                        vo3[:, g * 8:(g + 1) * 8, 0:64],
                        pv.rearrange("p (j c) -> p j c", c=64)[:, :, :])

            # ---------------- main attention loop ----------------
            with tc.tile_pool(name="ps1", bufs=1, space="PSUM") as ps1p, \
                 tc.tile_pool(name="ps2", bufs=2, space="PSUM") as ps2p, \
                 tc.tile_pool(name="psav", bufs=1, space="PSUM") as psavp, \
                 tc.tile_pool(name="pstiny", bufs=1, space="PSUM") as pstp, \
                 tc.tile_pool(name="ut", bufs=3) as utp, \
                 tc.tile_pool(name="Upool", bufs=2) as up, \
                 tc.tile_pool(name="sm", bufs=2) as smp:
                for c in range(4):
                    sl = slice(c * 512, (c + 1) * 512)
                    # ---- pass1: scores^T + exp + (attn_un @ [v|ones]) ----
                    pav = psavp.tile([65, 512], f32, tag="av")
                    for g in range(16):  # 4 t-tiles per batch
                        p1 = ps1p.tile([128, 2048], f32, tag="s1")
                        for u in range(4):
                            j = g * 4 + u
                            nc.tensor.matmul(
                                p1[:, u * 512:(u + 1) * 512],
                                mmt(kT[0:64, j * 128:(j + 1) * 128]),
                                mmt(qT[0:64, sl]))
                        ut = utp.tile([128, 2048], mm_store_dt, tag="ut")
                        nc.scalar.activation(ut[:], p1[:], AF.Exp, scale=float(SCALE))
                        for u in range(4):
                            j = g * 4 + u
                            nc.tensor.matmul(
                                pav[:],
                                mmt(vo[:, j * 65:j * 65 + 65]),
                                mmt(ut[:, u * 512:(u + 1) * 512]),
                                start=(j == 0), stop=(j == 63))
                    # ---- tail: rowsums -> -ln(r); normalize out^T ----
                    oun = smp.tile([65, 512], f32, tag="oun")
                    nc.vector.tensor_copy(oun[:], pav[:])
                    rec = smp.tile([1, 512], f32, tag="rec")
                    nc.vector.reciprocal(rec[:], oun[64:65, :])
                    ptr = pstp.tile([128, 512], f32, tag="tiny")
                    for k in range(4):
                        nc.tensor.transpose(
                            ptr[:, k:k + 1],
                            rec[0:1, k * 128:(k + 1) * 128], ident[0:1, 0:1])
                    # ln(1/r) == -ln(r): the softmax bias
                    nc.scalar.activation(
                        nlr[:, c * 4:(c + 1) * 4], ptr[:, 0:4], AF.Ln)
                    pbc = pstp.tile([128, 512], f32, tag="tiny")
                    nc.tensor.matmul(pbc[0:64, :], ones_row[0:1, 0:64], rec[:])
                    nc.vector.tensor_mul(outT[:, sl], oun[0:64, :], pbc[0:64, :])

                    # ---- pass2: scores [s,t] + normalized exp + DMA out ----
                    for st in range(4):
                        s_tile = c * 4 + st
                        U = up.tile([128, S], f32, tag="U")
                        for tt in range(16):
                            p2 = ps2p.tile([128, 512], f32, tag="s2")
                            nc.tensor.matmul(
                                p2[:],
                                mmt(qT[0:64, s_tile * 128:(s_tile + 1) * 128]),
                                mmt(kT[0:64, tt * 512:(tt + 1) * 512]))
                            nc.scalar.activation(
                                U[:, tt * 512:(tt + 1) * 512], p2[:], AF.Exp,
                                scale=SCALE, bias=nlr[:, s_tile:s_tile + 1])
                        nc.sync.dma_start(
                            attn_p[s_tile * 128:(s_tile + 1) * 128, :], U[:])

            # ---------------- MLP ----------------
            with tc.tile_pool(name="mlp", bufs=1) as mp, \
                 tc.tile_pool(name="mlp_ps", bufs=1, space="PSUM") as mps, \
                 tc.tile_pool(name="mlp_ps2", bufs=2, space="PSUM") as mps2:
                # normalized out^T -> [e, n] layout: e = 64*h + d, s = 8n + h
                minT = mp.tile([128, 4 * 256], f32, tag="minT")
                outT3 = outT.rearrange("d (n h) -> d h n", h=8)
                minT3 = minT.rearrange("p (k n) -> p k n", n=256)
                for k in range(4):
                    for p_ in range(2):
                        h = 2 * k + p_
                        nc.any.tensor_copy(
                            minT3[64 * p_:64 * (p_ + 1), k:k + 1, :],
                            outT3[:, h:h + 1, :])

                # w1 [2E, E] -> w1T [e, f] tiles; w2 [E, 2E] -> w2T [f, e'] tiles
                w1s = mp.tile([128, 8 * 512], f32, tag="w1s")
                nc.sync.dma_start(
                    w1s.rearrange("p (i e) -> p i e", i=8),
                    w1_t.rearrange("(i p) e -> p i e", p=128))
                w2s = mp.tile([128, 4 * 1024], f32, tag="w2s")
                nc.sync.dma_start(
                    w2s.rearrange("p (i f) -> p i f", i=4),
                    w2_t.rearrange("(i p) f -> p i f", p=128))
                w1T = mp.tile([128, 4 * 1024], f32, tag="w1T")
                for kf in range(8):
                    pt = mps.tile([128, 512], f32, tag="wtr")
                    for ke in range(4):
                        nc.tensor.transpose(
                            pt[:, ke * 128:(ke + 1) * 128],
                            w1s[:, kf * 512 + ke * 128:kf * 512 + (ke + 1) * 128],
                            ident[:, :])
                    # pt cols: ke-major [4 x 128 f-rows] -> scatter to w1T
                    nc.any.tensor_copy(
                        w1T.rearrange("p (ke f) -> p ke f", ke=4)
                           [:, :, kf * 128:(kf + 1) * 128],
                        pt.rearrange("p (ke f) -> p ke f", ke=4)[:, :, :])
                w2T = mp.tile([128, 8 * 512], f32, tag="w2T")
                for ke in range(4):
                    pt = mps.tile([128, 1024], f32, tag="wtr2")
                    for kf in range(8):
                        nc.tensor.transpose(
                            pt[:, kf * 128:(kf + 1) * 128],
                            w2s[:, ke * 1024 + kf * 128:ke * 1024 + (kf + 1) * 128],
                            ident[:, :])
                    nc.any.tensor_copy(
                        w2T.rearrange("p (kf e) -> p kf e", kf=8)
                           [:, :, ke * 128:(ke + 1) * 128],
                        pt.rearrange("p (kf e) -> p kf e", kf=8)[:, :, :])

                # biases
                brow = mp.tile([1, 2 * EMBED + EMBED], f32, tag="brow")
                nc.sync.dma_start(
                    brow[:, 0:2 * EMBED], b1_t.rearrange("(o f) -> o f", o=1))
                nc.sync.dma_start(
                    brow[:, 2 * EMBED:], b2_t.rearrange("(o f) -> o f", o=1))
                b1T = mp.tile([128, 8], f32, tag="b1T")
                ptb = mps.tile([128, 512], f32, tag="wtr")
                for kf in range(8):
                    nc.tensor.transpose(
                        ptb[:, kf:kf + 1],
                        brow[0:1, kf * 128:(kf + 1) * 128], ident[0:1, 0:1])
                nc.any.tensor_copy(b1T[:], ptb[:, 0:8])

                # h1^T = gelu(w1T.T @ minT + b1)  [f, n]
                h1T = mp.tile([128, 8 * 256], f32, tag="h1T")
                for kf in range(8):
                    ph = mps2.tile([128, 256], f32, tag="h1")
                    for ke in range(4):
                        nc.tensor.matmul(
                            ph[:],
                            w1T[:, ke * 1024 + kf * 128:ke * 1024 + (kf + 1) * 128],
                            minT[:, ke * 256:(ke + 1) * 256],
                            start=(ke == 0), stop=(ke == 3))
                    nc.scalar.activation(
                        h1T[:, kf * 256:(kf + 1) * 256], ph[:], AF.Gelu,
                        bias=b1T[:, kf:kf + 1])

                # y = h1T.T @ w2T + b2   [n, e]
                ys = mp.tile([128, 2 * 512], f32, tag="ys")
                for nch in range(2):
                    py = mps2.tile([128, 512], f32, tag="y")
                    for kf in range(8):
                        nc.tensor.matmul(
                            py[:],
                            h1T[:, kf * 256 + nch * 128:kf * 256 + (nch + 1) * 128],
                            w2T[:, kf * 512:(kf + 1) * 512],
                            start=(kf == 0), stop=False)
                    nc.tensor.matmul(
                        py[:], ones_row[0:1, :], brow[0:1, 2 * EMBED:],
                        start=False, stop=True)
                    nc.any.tensor_copy(ys[:, nch * 512:(nch + 1) * 512], py[:])
                nc.sync.dma_start(
                    y_p.rearrange("(c n) e -> n c e", c=2),
                    ys.rearrange("n (c e) -> n c e", c=2))

    nc.compile()
    return nc


def kernel(x, wq, wk, wv, w1, b1, w2, b2):
    from concourse.bass_utils import run_bass_kernel_spmd

    if "nc" not in _cache:
        _cache["nc"] = _build_nc()
    nc = _cache["nc"]

    x = np.ascontiguousarray(np.asarray(x, dtype=np.float32))
    common = {
        "wq": np.ascontiguousarray(np.asarray(wq, np.float32)),
        "wk": np.ascontiguousarray(np.asarray(wk, np.float32)),
        "wv": np.ascontiguousarray(np.asarray(wv, np.float32)),
        "w1": np.ascontiguousarray(np.asarray(w1, np.float32)),
        "b1": np.ascontiguousarray(np.asarray(b1, np.float32)),
        "w2": np.ascontiguousarray(np.asarray(w2, np.float32)),
        "b2": np.ascontiguousarray(np.asarray(b2, np.float32)),
    }
    in_maps = []
    for core in range(8):
        bi, qi = core // 4, core % 4
        in_maps.append({
            "x_full": x[bi],
            "x_q": np.ascontiguousarray(x[bi, qi * NQ:(qi + 1) * NQ]),
            **common,
        })

    res = run_bass_kernel_spmd(
        nc, in_maps, core_ids=list(range(8)),
        trace=bool(int(os.environ.get("BASS_KERNEL_TRACE", "0"))))
    _cache["last_result"] = res

    attn = np.empty((B, S, S), dtype=np.float32)
    out = np.empty((B, N, EMBED), dtype=np.float32)
    for core in range(8):
        bi, qi = core // 4, core % 4
        attn[bi, qi * SC:(qi + 1) * SC, :] = res.results[core]["attn_p"]
        out[bi, qi * NQ:(qi + 1) * NQ, :] = res.results[core]["y_p"]
    return attn, out


if __name__ == "__main__":
    rng = np.random.default_rng(0)
    ins = {
        "x": rng.standard_normal((B, N, EMBED), dtype=np.float32),
        "wq": rng.standard_normal((HD, HD), dtype=np.float32) * 0.02,
        "wk": rng.standard_normal((HD, HD), dtype=np.float32) * 0.02,
        "wv": rng.standard_normal((HD, HD), dtype=np.float32) * 0.02,
        "w1": rng.standard_normal((2 * EMBED, EMBED), dtype=np.float32) * 0.02,
        "b1": np.zeros(2 * EMBED, np.float32),
        "w2": rng.standard_normal((EMBED, 2 * EMBED), dtype=np.float32) * 0.02,
        "b2": np.zeros(EMBED, np.float32),
    }
    a, o = kernel(**ins)
    print("attn", a.shape, a.dtype, "out", o.shape, o.dtype)
